# revision 1
# baseline (speedup 1.0000x reference)
"""Multi-head attention (B=2, S=2048, D=1024, H=16) on 8 TRN2 NeuronCores.

Sharding: 8-way tensor-parallel over heads (2 heads/core), Megatron-style.
Each core:
  - holds X^T (feature-major) for all 4096 flat tokens (bf16),
  - computes Q^T/K^T (feature-major) and V (token-major) for its 2 heads,
  - computes scores S^T = K_h Q_h^T per 128-k-tile with two K=64 matmuls
    row-packed into the 128x128 PE array (one per head),
  - softmax: exp on ScalarE (scale=1/8 folded in, no max-subtraction --
    scores are ~N(0, 1/3) so exp is safe), denominator via a ones-column
    appended to V (M=65 matmul), division via reciprocal+partition_broadcast,
  - AllToAll over all 8 cores redistributes attention outputs from
    head-sharded to token-sharded (each core ends with all 1024 attention
    features for its 512-token chunk),
  - out-projection (full Wo^T) + bias for its token chunk.
Host only pre-transposes/slices inputs and re-assembles output slices.
"""
import numpy as np
import ml_dtypes

import concourse.bass as bass
import concourse.bacc as bacc
import concourse.tile as tile
import concourse.mybir as mybir
from concourse.bass_utils import run_bass_kernel_spmd
from concourse.bass_interp import get_hw_module

NCORES = 8
B, S, D = 2, 2048, 1024
H, HD = 16, 64
T = B * S                 # 4096 flat tokens
HPC = H // NCORES         # 2 heads per core
FPC = HPC * HD            # 128 features per core
TPC = T // NCORES         # 512-token output chunk per core
ND = D // 128             # 8 d-tiles
NTT = T // 128            # 32 token-tiles
NTC = T // 512            # 8 512-token chunks
NKT = S // 128            # 16 key tiles per batch
NQC = S // 512            # 4 query chunks per batch
SCALE = 1.0 / float(np.sqrt(HD))

BF16 = mybir.dt.bfloat16
F32 = mybir.dt.float32


def build():
    nc = bacc.Bacc("TRN2", target_bir_lowering=False, debug=False,
                   num_devices=NCORES)
    xt = nc.dram_tensor("xt", [D, T], BF16, kind="ExternalInput").ap()
    wqt = nc.dram_tensor("wqt", [D, FPC], BF16, kind="ExternalInput").ap()
    wkt = nc.dram_tensor("wkt", [D, FPC], BF16, kind="ExternalInput").ap()
    wvt = nc.dram_tensor("wvt", [D, FPC], BF16, kind="ExternalInput").ap()
    wot = nc.dram_tensor("wot", [D, D], BF16, kind="ExternalInput").ap()
    bq = nc.dram_tensor("bq", [FPC, 1], F32, kind="ExternalInput").ap()
    bk = nc.dram_tensor("bk", [FPC, 1], F32, kind="ExternalInput").ap()
    bv_row = nc.dram_tensor("bv_row", [1, FPC], BF16, kind="ExternalInput").ap()
    bo = nc.dram_tensor("bo", [D, 1], F32, kind="ExternalInput").ap()
    out_t = nc.dram_tensor("out_t", [D, TPC], F32, kind="ExternalOutput").ap()

    with tile.TileContext(nc) as tc:
        with (
            tc.tile_pool(name="wts", bufs=1) as wts,
            tc.tile_pool(name="acts", bufs=1) as acts,
            tc.tile_pool(name="p_pool", bufs=3) as p_pool,
            tc.tile_pool(name="div_pool", bufs=2) as div_pool,
            tc.tile_pool(name="fin_pool", bufs=2) as fin_pool,
            tc.tile_pool(name="mm_ps", bufs=4, space="PSUM") as mm_ps,
            tc.tile_pool(name="s_ps_pool", bufs=2, space="PSUM") as s_ps_pool,
            tc.tile_pool(name="dram", bufs=1, space="DRAM") as dram,
        ):
            # ---------------- load weights / biases / x ----------------
            wq_sb = wts.tile([128, ND, FPC], BF16, name="wq_sb")
            wk_sb = wts.tile([128, ND, FPC], BF16, name="wk_sb")
            wv_sb = wts.tile([128, ND, FPC], BF16, name="wv_sb")
            wot_sb = wts.tile([128, ND, D], BF16, name="wot_sb")
            for d in range(ND):
                nc.sync.dma_start(out=wq_sb[:, d, :], in_=wqt[128 * d:128 * (d + 1), :])
                nc.sync.dma_start(out=wk_sb[:, d, :], in_=wkt[128 * d:128 * (d + 1), :])
                nc.sync.dma_start(out=wv_sb[:, d, :], in_=wvt[128 * d:128 * (d + 1), :])
                nc.sync.dma_start(out=wot_sb[:, d, :], in_=wot[128 * d:128 * (d + 1), :])
            bq_sb = wts.tile([FPC, 1], F32, name="bq_sb")
            bk_sb = wts.tile([FPC, 1], F32, name="bk_sb")
            bo_sb = wts.tile([128, ND], F32, name="bo_sb")
            nc.sync.dma_start(out=bq_sb[:], in_=bq[:])
            nc.sync.dma_start(out=bk_sb[:], in_=bk[:])
            for m in range(ND):
                nc.sync.dma_start(out=bo_sb[:, m:m + 1], in_=bo[128 * m:128 * (m + 1), :])
            bv_row_sb = wts.tile([1, FPC], BF16, name="bv_row_sb")
            nc.sync.dma_start(out=bv_row_sb[:], in_=bv_row[:])
            bv_bc = wts.tile([128, FPC], BF16, name="bv_bc")
            nc.gpsimd.partition_broadcast(bv_bc[:], bv_row_sb[:])

            xt_sb = acts.tile([128, ND, T], BF16, name="xt_sb")
            for d in range(ND):
                nc.sync.dma_start(out=xt_sb[:, d, :], in_=xt[128 * d:128 * (d + 1), :])

            # ---------------- Q/K projections (feature-major) ----------
            qt_sb = acts.tile([FPC, T], BF16, name="qt_sb")
            kt_sb = acts.tile([FPC, T], BF16, name="kt_sb")
            for w_sb, b_sb, dst in ((wk_sb, bk_sb, kt_sb), (wq_sb, bq_sb, qt_sb)):
                for tch in range(NTC):
                    ps = mm_ps.tile([128, 512], F32, tag="mm_ps", name="proj_ps")
                    for d in range(ND):
                        nc.tensor.matmul(
                            ps[:], w_sb[:, d, :],
                            xt_sb[:, d, 512 * tch:512 * (tch + 1)],
                            start=(d == 0), stop=(d == ND - 1))
                    nc.vector.tensor_scalar_add(
                        dst[:, 512 * tch:512 * (tch + 1)], ps[:], b_sb[:])

            # ---------------- V projection (token-major) + ones --------
            v_sb = acts.tile([128, NTT, HPC, HD + 1], BF16, name="v_sb")
            nc.vector.memset(v_sb[:, :, :, HD:HD + 1], 1.0)
            for tt in range(NTT):
                ps = mm_ps.tile([128, FPC], F32, tag="mm_ps", name="v_ps")
                for d in range(ND):
                    nc.tensor.matmul(
                        ps[:], xt_sb[:, d, 128 * tt:128 * (tt + 1)],
                        wv_sb[:, d, :],
                        start=(d == 0), stop=(d == ND - 1))
                nc.vector.tensor_tensor(
                    v_sb[:, tt, :, 0:HD],
                    ps.rearrange("p (h f) -> p h f", h=HPC),
                    bv_bc.rearrange("p (h f) -> p h f", h=HPC),
                    mybir.AluOpType.add)

            # ---------------- attention + A2A staging -------------------
            cc_in = dram.tile([NCORES * FPC, TPC], BF16, name="cc_in")
            cc_out = dram.tile([NCORES * FPC, TPC], BF16, name="cc_out")

            for b in range(B):
                for qc in range(NQC):
                    q0 = 2048 * b + 512 * qc
                    o_ps = [
                        mm_ps.tile([128, 512], F32, tag="mm_ps", name=f"o_ps{h}")
                        for h in range(HPC)
                    ]
                    for k in range(NKT):
                        k0 = 2048 * b + 128 * k
                        s_ps = s_ps_pool.tile([128, HPC, 512], F32, name="s_ps")
                        for h in range(HPC):
                            nc.tensor.matmul(
                                s_ps[:, h, :],
                                kt_sb[64 * h:64 * (h + 1), k0:k0 + 128],
                                qt_sb[64 * h:64 * (h + 1), q0:q0 + 512],
                                start=True, stop=True)
                        p_t = p_pool.tile([128, HPC, 512], BF16, name="p_t")
                        nc.scalar.activation(
                            p_t[:], s_ps[:],
                            mybir.ActivationFunctionType.Exp, scale=SCALE)
                        for h in range(HPC):
                            nc.tensor.matmul(
                                o_ps[h][0:HD + 1, :],
                                v_sb[:, NKT * b + k, h, :],
                                p_t[:, h, :],
                                start=(k == 0), stop=(k == NKT - 1))
                    j = NQC * b + qc
                    for h in range(HPC):
                        recip = div_pool.tile([1, 512], F32, name="recip")
                        nc.vector.reciprocal(recip[:], o_ps[h][HD:HD + 1, :])
                        rb = div_pool.tile([HD, 512], F32, name="rb")
                        nc.gpsimd.partition_broadcast(rb[:], recip[:])
                        avs = div_pool.tile([HD, 512], BF16, name="avs")
                        nc.vector.tensor_tensor(
                            avs[:], o_ps[h][0:HD, :], rb[:],
                            mybir.AluOpType.mult)
                        nc.sync.dma_start(
                            out=cc_in[FPC * j + HD * h: FPC * j + HD * (h + 1), :],
                            in_=avs[:])

            # ---------------- AllToAll ----------------------------------
            nc.gpsimd.collective_compute(
                "AllToAll", mybir.AluOpType.bypass,
                replica_groups=[list(range(NCORES))],
                ins=[cc_in.opt()], outs=[cc_out.opt()])
            at_full = acts.tile([128, NCORES, TPC], BF16, name="at_full")
            for jj in range(NCORES):
                nc.sync.dma_start(
                    out=at_full[:, jj, :],
                    in_=cc_out[FPC * jj:FPC * (jj + 1), :])

            # ---------------- out projection ----------------------------
            for m in range(ND):
                ps = mm_ps.tile([128, 512], F32, tag="mm_ps", name="f_ps")
                for d in range(ND):
                    nc.tensor.matmul(
                        ps[:], wot_sb[:, d, 128 * m:128 * (m + 1)],
                        at_full[:, d, :],
                        start=(d == 0), stop=(d == ND - 1))
                fin = fin_pool.tile([128, TPC], F32, name="fin")
                nc.vector.tensor_scalar_add(fin[:], ps[:], bo_sb[:, m:m + 1])
                nc.sync.dma_start(out=out_t[128 * m:128 * (m + 1), :], in_=fin[:])

    nc.compile()
    nc.m = get_hw_module(nc.m)
    return nc


_NC_CACHE = None


def _get_nc():
    global _NC_CACHE
    if _NC_CACHE is None:
        _NC_CACHE = build()
    return _NC_CACHE


def _make_in_maps(x, Wq, bq, Wk, bk, Wv, bv, Wo, bo):
    bf16 = ml_dtypes.bfloat16
    x = np.asarray(x, np.float32)
    xt = np.ascontiguousarray(x.reshape(T, D).T).astype(bf16)
    wot = np.ascontiguousarray(np.asarray(Wo, np.float32).T).astype(bf16)
    bo_col = np.asarray(bo, np.float32).reshape(D, 1)
    in_maps = []
    for c in range(NCORES):
        hs = slice(FPC * c, FPC * (c + 1))
        in_maps.append({
            "xt": xt,
            "wqt": np.ascontiguousarray(np.asarray(Wq, np.float32)[hs, :].T).astype(bf16),
            "wkt": np.ascontiguousarray(np.asarray(Wk, np.float32)[hs, :].T).astype(bf16),
            "wvt": np.ascontiguousarray(np.asarray(Wv, np.float32)[hs, :].T).astype(bf16),
            "wot": wot,
            "bq": np.asarray(bq, np.float32)[hs].reshape(FPC, 1),
            "bk": np.asarray(bk, np.float32)[hs].reshape(FPC, 1),
            "bv_row": np.asarray(bv, np.float32)[hs].reshape(1, FPC).astype(bf16),
            "bo": bo_col,
        })
    return in_maps


def run_on_hw(in_maps, trace=False):
    nc = _get_nc()
    return run_bass_kernel_spmd(nc, in_maps, list(range(NCORES)), trace=trace)


def _assemble(results):
    out = np.empty((T, D), np.float32)
    for c in range(NCORES):
        out[TPC * c:TPC * (c + 1), :] = results[c]["out_t"].T
    return out.reshape(B, S, D)


def kernel(x, Wq, bq, Wk, bk, Wv, bv, Wo, bo):
    in_maps = _make_in_maps(x, Wq, bq, Wk, bk, Wv, bv, Wo, bo)
    res = run_on_hw(in_maps, trace=False)
    return _assemble(res.results)


# revision 4
# speedup vs baseline: 1.0622x; 1.0622x over previous
"""Multi-head attention (B=2, S=2048, D=1024, H=16) on 8 TRN2 NeuronCores.

Sharding: 8-way tensor-parallel over heads (2 heads/core), Megatron-style.
Each core:
  - holds X^T (feature-major) for all 4096 flat tokens (bf16),
  - computes Q^T/K^T (feature-major) and V (token-major) for its 2 heads,
  - computes scores S^T = K_h Q_h^T per 128-k-tile with two K=64 matmuls
    row-packed into the 128x128 PE array (one per head),
  - softmax: exp on ScalarE (scale=1/8 folded in, no max-subtraction --
    scores are ~N(0, 1/3) so exp is safe), denominator via a ones-column
    appended to V (M=65 matmul), division via fast reciprocal +
    partition_broadcast,
  - AllToAll over all 8 cores redistributes attention outputs from
    head-sharded to token-sharded (each core ends with all 1024 attention
    features for its 512-token chunk),
  - out-projection (full Wo^T) + bias for its token chunk.
Host only pre-transposes/slices inputs and re-assembles output slices.

Program order is arranged so batch-0 attention starts as soon as the
batch-0 half of X^T and the first K/Q/V projection chunks land; the rest
of the projections overlap attention via Tile's dependency scheduling.
"""
import numpy as np
import ml_dtypes

import concourse.bass as bass
import concourse.bacc as bacc
import concourse.tile as tile
import concourse.mybir as mybir
from concourse.bass_utils import run_bass_kernel_spmd
from concourse.bass_interp import get_hw_module

NCORES = 8
B, S, D = 2, 2048, 1024
H, HD = 16, 64
T = B * S                 # 4096 flat tokens
HPC = H // NCORES         # 2 heads per core
FPC = HPC * HD            # 128 features per core
TPC = T // NCORES         # 512-token output chunk per core
ND = D // 128             # 8 d-tiles
NTT = T // 128            # 32 token-tiles
NTC = T // 512            # 8 512-token chunks
NKT = S // 128            # 16 key tiles per batch
NQC = S // 512            # 4 query chunks per batch
SCALE = 1.0 / float(np.sqrt(HD))

BF16 = mybir.dt.bfloat16
F32 = mybir.dt.float32


def build():
    nc = bacc.Bacc("TRN2", target_bir_lowering=False, debug=False,
                   num_devices=NCORES)
    xt = nc.dram_tensor("xt", [D, T], BF16, kind="ExternalInput").ap()
    wqt = nc.dram_tensor("wqt", [D, FPC], BF16, kind="ExternalInput").ap()
    wkt = nc.dram_tensor("wkt", [D, FPC], BF16, kind="ExternalInput").ap()
    wvt = nc.dram_tensor("wvt", [D, FPC], BF16, kind="ExternalInput").ap()
    wot = nc.dram_tensor("wot", [D, D], BF16, kind="ExternalInput").ap()
    bq = nc.dram_tensor("bq", [FPC, 1], F32, kind="ExternalInput").ap()
    bk = nc.dram_tensor("bk", [FPC, 1], F32, kind="ExternalInput").ap()
    bv_row = nc.dram_tensor("bv_row", [1, FPC], BF16, kind="ExternalInput").ap()
    bo = nc.dram_tensor("bo", [D, 1], F32, kind="ExternalInput").ap()
    out_t = nc.dram_tensor("out_t", [D, TPC], F32, kind="ExternalOutput").ap()

    with tile.TileContext(nc) as tc:
        with (
            tc.tile_pool(name="wts", bufs=1) as wts,
            tc.tile_pool(name="acts", bufs=1) as acts,
            tc.tile_pool(name="p_pool", bufs=3) as p_pool,
            tc.tile_pool(name="div_pool", bufs=2) as div_pool,
            tc.tile_pool(name="fin_pool", bufs=2) as fin_pool,
            tc.tile_pool(name="mm_ps", bufs=4, space="PSUM") as mm_ps,
            tc.tile_pool(name="s_ps_pool", bufs=2, space="PSUM") as s_ps_pool,
            tc.tile_pool(name="dram", bufs=1, space="DRAM") as dram,
        ):
            # ---- x (batch-0 half first, sync queue) + small weights (scalar
            # ---- queue) land in parallel; wot/bo (tail-only) go last.
            xt_sb = acts.tile([128, ND, T], BF16, name="xt_sb")
            for d in range(ND):
                nc.sync.dma_start(out=xt_sb[:, d, 0:S], in_=xt[128 * d:128 * (d + 1), 0:S])
            wq_sb = wts.tile([128, ND, FPC], BF16, name="wq_sb")
            wk_sb = wts.tile([128, ND, FPC], BF16, name="wk_sb")
            wv_sb = wts.tile([128, ND, FPC], BF16, name="wv_sb")
            for d in range(ND):
                nc.scalar.dma_start(out=wk_sb[:, d, :], in_=wkt[128 * d:128 * (d + 1), :])
            for d in range(ND):
                nc.scalar.dma_start(out=wq_sb[:, d, :], in_=wqt[128 * d:128 * (d + 1), :])
            for d in range(ND):
                nc.scalar.dma_start(out=wv_sb[:, d, :], in_=wvt[128 * d:128 * (d + 1), :])
            bq_sb = wts.tile([FPC, 1], F32, name="bq_sb")
            bk_sb = wts.tile([FPC, 1], F32, name="bk_sb")
            bv_row_sb = wts.tile([1, FPC], BF16, name="bv_row_sb")
            nc.scalar.dma_start(out=bk_sb[:], in_=bk[:])
            nc.scalar.dma_start(out=bq_sb[:], in_=bq[:])
            nc.scalar.dma_start(out=bv_row_sb[:], in_=bv_row[:])
            bv_bc = wts.tile([128, FPC], BF16, name="bv_bc")
            nc.gpsimd.partition_broadcast(bv_bc[:], bv_row_sb[:])
            for d in range(ND):
                nc.sync.dma_start(out=xt_sb[:, d, S:T], in_=xt[128 * d:128 * (d + 1), S:T])
            # tail-only loads, off the critical queues
            wot_sb = wts.tile([128, ND, D], BF16, name="wot_sb")
            bo_sb = wts.tile([128, ND], F32, name="bo_sb")
            for d in range(ND):
                nc.gpsimd.dma_start(out=wot_sb[:, d, :], in_=wot[128 * d:128 * (d + 1), :])
            for m in range(ND):
                nc.gpsimd.dma_start(out=bo_sb[:, m:m + 1], in_=bo[128 * m:128 * (m + 1), :])

            # warm up the ACT exp table while DMAs land
            warm_sb = wts.tile([1, 8], F32, name="warm_sb")
            nc.vector.memset(warm_sb[:], 0.0)
            nc.scalar.activation(warm_sb[:], warm_sb[:],
                                 mybir.ActivationFunctionType.Exp, scale=1.0)

            qt_sb = acts.tile([FPC, T], BF16, name="qt_sb")
            kt_sb = acts.tile([FPC, T], BF16, name="kt_sb")
            v_sb = acts.tile([128, NTT, HPC, HD + 1], BF16, name="v_sb")
            nc.vector.memset(v_sb[:, :, :, HD:HD + 1], 1.0)

            def proj_qk(w_sb, b_sb, dst, tch):
                ps = mm_ps.tile([128, 512], F32, tag="mm_ps", name="proj_ps")
                for d in range(ND):
                    nc.tensor.matmul(
                        ps[:], w_sb[:, d, :],
                        xt_sb[:, d, 512 * tch:512 * (tch + 1)],
                        start=(d == 0), stop=(d == ND - 1))
                nc.vector.tensor_scalar_add(
                    dst[:, 512 * tch:512 * (tch + 1)], ps[:], b_sb[:])

            def proj_v(tt):
                ps = mm_ps.tile([128, FPC], F32, tag="mm_ps", name="v_ps")
                for d in range(ND):
                    nc.tensor.matmul(
                        ps[:], xt_sb[:, d, 128 * tt:128 * (tt + 1)],
                        wv_sb[:, d, :],
                        start=(d == 0), stop=(d == ND - 1))
                nc.vector.tensor_tensor(
                    v_sb[:, tt, :, 0:HD],
                    ps.rearrange("p (h f) -> p h f", h=HPC),
                    bv_bc.rearrange("p (h f) -> p h f", h=HPC),
                    mybir.AluOpType.add)

            cc_in = dram.tile([NCORES * FPC, TPC], BF16, name="cc_in")
            cc_out = dram.tile([NCORES * FPC, TPC], BF16, name="cc_out")

            def attention(b, qc):
                q0 = 2048 * b + 512 * qc
                o_ps = [
                    mm_ps.tile([128, 512], F32, tag="mm_ps", name=f"o_ps{h}")
                    for h in range(HPC)
                ]
                for k in range(NKT):
                    k0 = 2048 * b + 128 * k
                    s_ps = s_ps_pool.tile([128, HPC, 512], F32, name="s_ps")
                    for h in range(HPC):
                        nc.tensor.matmul(
                            s_ps[:, h, :],
                            kt_sb[64 * h:64 * (h + 1), k0:k0 + 128],
                            qt_sb[64 * h:64 * (h + 1), q0:q0 + 512],
                            start=True, stop=True)
                    p_t = p_pool.tile([128, HPC, 512], BF16, name="p_t")
                    nc.scalar.activation(
                        p_t[:], s_ps[:],
                        mybir.ActivationFunctionType.Exp, scale=SCALE)
                    for h in range(HPC):
                        nc.tensor.matmul(
                            o_ps[h][0:HD + 1, :],
                            v_sb[:, NKT * b + k, h, :],
                            p_t[:, h, :],
                            start=(k == 0), stop=(k == NKT - 1))
                j = NQC * b + qc
                for h in range(HPC):
                    den_sb = div_pool.tile([1, 512], F32, name="den_sb")
                    nc.vector.tensor_copy(den_sb[:], o_ps[h][HD:HD + 1, :])
                    recip = div_pool.tile([1, 512], F32, name="recip")
                    nc.vector.reciprocal_approx_fast(recip[:], den_sb[:])
                    rb = div_pool.tile([HD, 512], F32, name="rb")
                    nc.gpsimd.partition_broadcast(rb[:], recip[:])
                    avs = div_pool.tile([HD, 512], BF16, name="avs")
                    nc.vector.tensor_tensor(
                        avs[:], o_ps[h][0:HD, :], rb[:],
                        mybir.AluOpType.mult)
                    nc.sync.dma_start(
                        out=cc_in[FPC * j + HD * h: FPC * j + HD * (h + 1), :],
                        in_=avs[:])

            # ---- batch 0: minimal prefix, then attention overlapped with
            # ---- the remaining projections
            for tch in range(4):
                proj_qk(wk_sb, bk_sb, kt_sb, tch)
            proj_qk(wq_sb, bq_sb, qt_sb, 0)
            for tt in range(16):
                proj_v(tt)
            attention(0, 0)
            for qc in range(1, NQC):
                proj_qk(wq_sb, bq_sb, qt_sb, qc)
                attention(0, qc)
            # ---- batch 1
            for tch in range(4, 8):
                proj_qk(wk_sb, bk_sb, kt_sb, tch)
            proj_qk(wq_sb, bq_sb, qt_sb, 4)
            for tt in range(16, 32):
                proj_v(tt)
            attention(1, 0)
            for qc in range(1, NQC):
                proj_qk(wq_sb, bq_sb, qt_sb, 4 + qc)
                attention(1, qc)

            # ---------------- AllToAll ----------------------------------
            nc.gpsimd.collective_compute(
                "AllToAll", mybir.AluOpType.bypass,
                replica_groups=[list(range(NCORES))],
                ins=[cc_in.opt()], outs=[cc_out.opt()])
            at_full = acts.tile([128, NCORES, TPC], BF16, name="at_full")
            for jj in range(NCORES):
                nc.sync.dma_start(
                    out=at_full[:, jj, :],
                    in_=cc_out[FPC * jj:FPC * (jj + 1), :])

            # ---------------- out projection ----------------------------
            for m in range(ND):
                ps = mm_ps.tile([128, 512], F32, tag="mm_ps", name="f_ps")
                for d in range(ND):
                    nc.tensor.matmul(
                        ps[:], wot_sb[:, d, 128 * m:128 * (m + 1)],
                        at_full[:, d, :],
                        start=(d == 0), stop=(d == ND - 1))
                fin = fin_pool.tile([128, TPC], F32, name="fin")
                nc.vector.tensor_scalar_add(fin[:], ps[:], bo_sb[:, m:m + 1])
                nc.sync.dma_start(out=out_t[128 * m:128 * (m + 1), :], in_=fin[:])

    nc.compile()
    nc.m = get_hw_module(nc.m)
    return nc


_NC_CACHE = None


def _get_nc():
    global _NC_CACHE
    if _NC_CACHE is None:
        _NC_CACHE = build()
    return _NC_CACHE


def _make_in_maps(x, Wq, bq, Wk, bk, Wv, bv, Wo, bo):
    bf16 = ml_dtypes.bfloat16
    x = np.asarray(x, np.float32)
    xt = np.ascontiguousarray(x.reshape(T, D).T).astype(bf16)
    wot = np.ascontiguousarray(np.asarray(Wo, np.float32).T).astype(bf16)
    bo_col = np.asarray(bo, np.float32).reshape(D, 1)
    in_maps = []
    for c in range(NCORES):
        hs = slice(FPC * c, FPC * (c + 1))
        in_maps.append({
            "xt": xt,
            "wqt": np.ascontiguousarray(np.asarray(Wq, np.float32)[hs, :].T).astype(bf16),
            "wkt": np.ascontiguousarray(np.asarray(Wk, np.float32)[hs, :].T).astype(bf16),
            "wvt": np.ascontiguousarray(np.asarray(Wv, np.float32)[hs, :].T).astype(bf16),
            "wot": wot,
            "bq": np.asarray(bq, np.float32)[hs].reshape(FPC, 1),
            "bk": np.asarray(bk, np.float32)[hs].reshape(FPC, 1),
            "bv_row": np.asarray(bv, np.float32)[hs].reshape(1, FPC).astype(bf16),
            "bo": bo_col,
        })
    return in_maps


def run_on_hw(in_maps, trace=False):
    nc = _get_nc()
    return run_bass_kernel_spmd(nc, in_maps, list(range(NCORES)), trace=trace)


def _assemble(results):
    out = np.empty((T, D), np.float32)
    for c in range(NCORES):
        out[TPC * c:TPC * (c + 1), :] = results[c]["out_t"].T
    return out.reshape(B, S, D)


def kernel(x, Wq, bq, Wk, bk, Wv, bv, Wo, bo):
    in_maps = _make_in_maps(x, Wq, bq, Wk, bk, Wv, bv, Wo, bo)
    res = run_on_hw(in_maps, trace=False)
    return _assemble(res.results)
